# revision 16
# baseline (speedup 1.0000x reference)
"""DNDT (deep neural decision tree) forward kernel for 8 Trainium2 NeuronCores.

Math (per batch row b of 16384):
  h[f,j]   = (x[b,f] * W[j] + bias[f,j]) / t,  W = [1..4], bias = cumsum([0,-sorted_cuts])
  bins     = softmax_j(h)                       # [6, 4]
  leaf     = kron(bins[0], ..., bins[5])        # [4096]
  out[b]   = leaf @ leaf_score                  # [10]

Device algorithm (pure data parallel, 2048 rows/core, batch-major layout
[128 partitions x 16 rows-per-partition x ...]):
  * softmax shift uses the analytic bound g(x) = (x + 3*relu(x))/t instead of a
    max-reduce (softmax is shift invariant; h - g <= 0 so exp can't overflow),
    folded as  h' = x*(W[j]-1)/t + bias[f,j]/t - 3/t*relu(x).
  * unnormalized bins E = exp(h'); the softmax normalizer 1/prod_f(sum_j E)
    is folded into bins4 before the p45 kron (one tiny [.,4] multiply).
  * leaf is never materialized: leaf = p0123 (x) p45 with p0123 = bins0*bins1*
    bins2*bins3 kron (256), p45 = bins4*bins5 kron (16).  Then
      out[b,c] = sum_v p45[b,v] * C[b,c,v],   C = p0123 @ S2,
    where S2[u, c*16+v] = leaf_score[u*16+v, c] - a [.,256]@[256,160] matmul
    instead of [.,4096]@[4096,10] (16x fewer PE MACs + 26x less DVE build).
  * all DVE element-wise work stays fp32: measured on HW, bf16 TENSOR_TENSOR
    with kron-style broadcast APs runs ~2x SLOWER than fp32, and gpsimd has
    a ~0.5-1us fixed per-instruction overhead, so the DVE does all of it.
  * p0123 is transposed 128x128 at a time through the PE in float32r (fast
    single-pass streaming); the PSUM->SBUF bounce converts to bf16 so the
    [.,256]@[256,160] matmuls run with bf16 weights/scores (half LDWEIGHTS
    time, ~1e-3 rounding vs the 2e-2 tolerance).
  * a burst of junk fp32 matmuls on a memset tile keeps the PE busy from
    ~7.4us (before any input lands) so the HAM clock gate (1.2 -> 2.4 GHz)
    is open when the real matmuls arrive; sized to drain right as the first
    kron quarter is transposable.
  * input DMAs are split across both HWDGE queues (Sync: x, Activation:
    consts+scores+identity) so x - the critical input - lands first.
"""

import numpy as np

import concourse.bass as bass
import concourse.tile as tile
from concourse import bacc, mybir
from concourse.bass_utils import run_bass_kernel_spmd

N_CORES = 8
B = 16384
BC = B // N_CORES          # rows per core = 2048
P = 128                    # partitions
M = BC // P                # rows per partition = 16
NCHUNK = 2                 # pipeline chunks
CHM = M // NCHUNK          # rows per partition per chunk = 8
F32 = mybir.dt.float32
F32R = mybir.dt.float32r
BF16 = mybir.dt.bfloat16
N_WARM = 11                # junk matmuls to warm the PE clock gate


def _build_nc(neg3invt):
    nc = bacc.Bacc("TRN2", target_bir_lowering=False, debug=False,
                   num_devices=N_CORES)
    xd = nc.dram_tensor("x", [P, M * 6], F32, kind="ExternalInput")
    cstd = nc.dram_tensor("cst", [P, 2 * 24], F32, kind="ExternalInput")
    s2d = nc.dram_tensor("s2", [256, 160], BF16, kind="ExternalInput")
    idd = nc.dram_tensor("ident", [P, P], F32R, kind="ExternalInput")
    od = nc.dram_tensor("o", [P, M * 10], F32, kind="ExternalOutput")

    with tile.TileContext(nc) as tc:
        with tc.tile_pool(name="consts", bufs=1) as consts, \
             tc.tile_pool(name="work", bufs=2) as work, \
             tc.tile_pool(name="atp", bufs=8) as atp, \
             tc.tile_pool(name="ps_t", bufs=4, space="PSUM") as ps_t, \
             tc.tile_pool(name="ps_c", bufs=4, space="PSUM") as ps_c:
            # Junk matmuls on a memset tile keep the PE busy from the very
            # start of the kernel (before inputs land) so the HAM clock gate
            # is fully open when the real matmuls arrive.  fp32 (double-pass)
            # on purpose: more PE-busy cycles per instruction.  They rotate
            # through the multi-buffer transpose PSUM pool so they run dense
            # back-to-back (a single-buffer WAR chain leaves ~300ns gaps,
            # which keeps the clock gate closed AND delays the real work).
            junk = consts.tile([P, P], F32)
            nc.gpsimd.memset(junk[:], 1.0)
            for _ in range(N_WARM):
                wps = ps_t.tile([P, 2, P], F32R, tag="tp")
                nc.tensor.matmul(wps[:, 0, :].bitcast(F32), lhsT=junk[:],
                                 rhs=junk[:], start=True, stop=True)
            x_st = consts.tile([P, M, 6], F32)
            nc.sync.dma_start(out=x_st[:], in_=xd[:].rearrange("p (i f) -> p i f", i=M))
            cst_st = consts.tile([P, 2, 6, 4], F32)
            nc.scalar.dma_start(out=cst_st[:].rearrange("p k f j -> p (k f j)"),
                                in_=cstd[:])
            s2_sb = consts.tile([P, 2, 160], BF16)
            nc.scalar.dma_start(out=s2_sb[:], in_=s2d[:].rearrange("(k p) n -> p k n", p=P))
            ident = consts.tile([P, P], F32R)
            nc.scalar.dma_start(out=ident[:], in_=idd[:])

            for c in range(NCHUNK):
                xv = x_st[:, c * CHM:(c + 1) * CHM, :]
                # m = 3/t * relu(x) on ACT (Relu with positive scale commutes)
                m3 = work.tile([P, CHM, 6], F32, tag="m3")
                nc.scalar.activation(m3[:].rearrange("p i f -> p (i f)"),
                                     xv.rearrange("p i f -> p (i f)"),
                                     mybir.ActivationFunctionType.Relu,
                                     scale=-neg3invt)
                H = work.tile([P, CHM, 6, 4], F32, tag="H")
                nc.vector.tensor_mul(H[:], xv[:, :, :, None].broadcast_to((P, CHM, 6, 4)),
                                     cst_st[:, 0:1, :, :].broadcast_to((P, CHM, 6, 4)))
                nc.vector.tensor_add(H[:], H[:],
                                     cst_st[:, 1:2, :, :].broadcast_to((P, CHM, 6, 4)))
                nc.vector.tensor_sub(H[:], H[:],
                                     m3[:, :, :, None].broadcast_to((P, CHM, 6, 4)))
                E = work.tile([P, CHM, 6, 4], F32, tag="E")
                nc.scalar.activation(E[:].rearrange("p i f j -> p (i f j)"),
                                     H[:].rearrange("p i f j -> p (i f j)"),
                                     mybir.ActivationFunctionType.Exp)

                # p01/p23 krons (TensorTensor APs are limited to 3 free dims,
                # so the two feature-pairs can't fuse into one instruction)
                pairs = work.tile([P, CHM, 2, 16], F32, tag="pairs")
                for g, (fa, fb) in enumerate(((0, 1), (2, 3))):
                    nc.vector.tensor_mul(
                        pairs[:, :, g, :].rearrange("p i (a b) -> p i a b", a=4),
                        E[:, :, fa, :, None].broadcast_to((P, CHM, 4, 4)),
                        E[:, :, fb, None, :].broadcast_to((P, CHM, 4, 4)))
                p01 = pairs[:, :, 0, :]
                p23 = pairs[:, :, 1, :]
                # softmax normalizer: zr = 1/prod_f(sum_j E), folded into
                # bins4 before the p45 kron
                Z = work.tile([P, CHM, 6], F32, tag="Z")
                nc.vector.tensor_reduce(Z[:], E[:], axis=mybir.AxisListType.X,
                                        op=mybir.AluOpType.add)
                zp = work.tile([P, CHM], F32, tag="zp")
                nc.vector.tensor_reduce(zp[:], Z[:], axis=mybir.AxisListType.X,
                                        op=mybir.AluOpType.mult)
                zr = work.tile([P, CHM, 1], F32, tag="zr")
                nc.vector.reciprocal(zr[:, :, 0], zp[:])
                E4z = work.tile([P, CHM, 4], F32, tag="E4z")
                nc.vector.tensor_mul(E4z[:], E[:, :, 4, :],
                                     zr[:].broadcast_to((P, CHM, 4)))
                p45 = work.tile([P, CHM, 1, 16], F32, tag="p45")
                nc.vector.tensor_mul(
                    p45[:, :, 0, :].rearrange("p i (a b) -> p i a b", a=4),
                    E4z[:, :, :, None].broadcast_to((P, CHM, 4, 4)),
                    E[:, :, 5, None, :].broadcast_to((P, CHM, 4, 4)))
                A = work.tile([P, CHM, 256], F32R, tag="A")
                q = CHM // 2
                for hh in range(2):
                    sl = slice(hh * q, (hh + 1) * q)
                    nc.vector.tensor_mul(
                        A[:, sl, :].rearrange("p i (a b) -> p i a b", a=16),
                        p01[:, sl, :, None].broadcast_to((P, q, 16, 16)),
                        p23[:, sl, None, :].broadcast_to((P, q, 16, 16)))

                D = work.tile([P, CHM, 10, 16], F32, tag="D")
                O = work.tile([P, CHM, 10], F32, tag="O")
                for pair in range(CHM // 2):
                    sl = slice(pair * 2, pair * 2 + 2)
                    cpp = ps_c.tile([P, 2, 160], F32, tag="cp")
                    for hhalf in range(2):
                        i = pair * 2 + hhalf
                        tp = ps_t.tile([P, 2, P], F32R, tag="tp")
                        for k in range(2):
                            nc.tensor.transpose(tp[:, k, :],
                                                A[:, i, k * P:(k + 1) * P],
                                                ident[:])
                        at2 = atp.tile([P, 2, P], BF16, tag="at")
                        nc.scalar.copy(out=at2[:], in_=tp[:].bitcast(F32))
                        nc.tensor.matmul(cpp[:, hhalf, :], lhsT=at2[:, 0, :],
                                         rhs=s2_sb[:, 0, :], start=True, stop=False)
                        nc.tensor.matmul(cpp[:, hhalf, :], lhsT=at2[:, 1, :],
                                         rhs=s2_sb[:, 1, :], start=False, stop=True)
                    nc.vector.tensor_mul(
                        D[:, sl],
                        cpp[:].rearrange("p i (c v) -> p i c v", c=10),
                        p45[:, sl].broadcast_to((P, 2, 10, 16)))
                    nc.vector.tensor_reduce(O[:, sl], D[:, sl],
                                            axis=mybir.AxisListType.X,
                                            op=mybir.AluOpType.add)
                hc = CHM // 2
                for hx in range(2):
                    nc.sync.dma_start(
                        out=od[:].rearrange("p (i c) -> p i c", i=M)
                        [:, c * CHM + hx * hc:c * CHM + (hx + 1) * hc, :],
                        in_=O[:, hx * hc:(hx + 1) * hc, :])
    nc.compile()
    return nc


_CACHE = {}


def _prep_inputs(x, cuts, leaf_score, temperature):
    import ml_dtypes
    x = np.ascontiguousarray(np.asarray(x, dtype=np.float32))
    cuts = np.asarray(cuts, dtype=np.float32)
    leaf_score = np.asarray(leaf_score, dtype=np.float32)
    invt = 1.0 / float(np.asarray(temperature).reshape(-1)[0])

    # host-side parameter prep (tiny)
    sc = np.sort(cuts, axis=1)
    bias = np.cumsum(np.concatenate([np.zeros((6, 1), np.float32), -sc], axis=1,
                                    dtype=np.float32), axis=1)          # [6,4]
    W = np.arange(1.0, 5.0, dtype=np.float32)
    w2 = np.tile(((W - 1.0) * invt)[None, :], (6, 1))                    # [6,4]
    bt = bias * invt                                                     # [6,4]
    cst = np.ascontiguousarray(np.broadcast_to(
        np.stack([w2, bt]).reshape(1, 48), (P, 48)).astype(np.float32))
    s2 = np.ascontiguousarray(
        leaf_score.reshape(256, 16, 10).transpose(0, 2, 1).reshape(256, 160)
    ).astype(ml_dtypes.bfloat16)
    ident = np.eye(P, dtype=np.float32)

    xs = x.reshape(N_CORES, P, M * 6)
    return invt, [{"x": xs[i], "cst": cst, "s2": s2, "ident": ident}
                  for i in range(N_CORES)]


def kernel(x, cuts, leaf_score, temperature):
    invt, in_maps = _prep_inputs(x, cuts, leaf_score, temperature)
    key = ("nc", float(invt))
    if key not in _CACHE:
        _CACHE[key] = _build_nc(-3.0 * invt)
        _CACHE["nc"] = _CACHE[key]
    nc = _CACHE[key]
    res = run_bass_kernel_spmd(nc, in_maps, list(range(N_CORES))).results
    out = np.concatenate([r["o"].reshape(BC, 10) for r in res], axis=0)
    return out.astype(np.float32)


# revision 19
# speedup vs baseline: 1.0994x; 1.0994x over previous
"""DNDT (deep neural decision tree) forward kernel for 8 Trainium2 NeuronCores.

Math (per batch row b of 16384):
  h[f,j]   = (x[b,f] * W[j] + bias[f,j]) / t,  W = [1..4], bias = cumsum([0,-sorted_cuts])
  bins     = softmax_j(h)                       # [6, 4]
  leaf     = kron(bins[0], ..., bins[5])        # [4096]
  out[b]   = leaf @ leaf_score                  # [10]

Device algorithm (pure data parallel, 2048 rows/core, batch-major layout
[128 partitions x 16 rows-per-partition x ...]):
  * softmax shift uses the analytic bound g(x) = (x + 3*relu(x))/t instead of a
    max-reduce (softmax is shift invariant; h - g <= 0 so exp can't overflow),
    folded as  h' = x*(W[j]-1)/t + bias[f,j]/t - 3/t*relu(x).
  * unnormalized bins E = exp(h'); the softmax normalizer 1/prod_f(sum_j E)
    is folded into bins4 before the p45 kron (one tiny [.,4] multiply).
  * leaf is never materialized: leaf = p0123 (x) p45 with p0123 = bins0*bins1*
    bins2*bins3 kron (256), p45 = bins4*bins5 kron (16).  Then
      out[b,c] = sum_v p45[b,v] * C[b,c,v],   C = p0123 @ S2,
    where S2[u, c*16+v] = leaf_score[u*16+v, c] - a [.,256]@[256,160] matmul
    instead of [.,4096]@[4096,10] (16x fewer PE MACs + 26x less DVE build).
  * all DVE element-wise work stays fp32: measured on HW, bf16 TENSOR_TENSOR
    with kron-style broadcast APs runs ~2x SLOWER than fp32, and gpsimd has
    a ~0.5-1us fixed per-instruction overhead, so the DVE does all of it.
  * p0123 is transposed 128x128 at a time through the PE in float32r (fast
    single-pass streaming); the PSUM->SBUF bounce converts to bf16 so the
    [.,256]@[256,160] matmuls run with bf16 weights/scores (half LDWEIGHTS
    time, ~1e-3 rounding vs the 2e-2 tolerance).
  * a burst of junk fp32 matmuls on a memset tile keeps the PE busy from
    ~7.4us (before any input lands) so the HAM clock gate (1.2 -> 2.4 GHz)
    is open when the real matmuls arrive; sized to drain right as the first
    kron quarter is transposable.
  * input DMAs are split across both HWDGE queues (Sync: x, Activation:
    consts+scores+identity) so x - the critical input - lands first.
"""

import numpy as np

import concourse.bass as bass
import concourse.tile as tile
from concourse import bacc, mybir
from concourse.bass_utils import run_bass_kernel_spmd

N_CORES = 8
B = 16384
BC = B // N_CORES          # rows per core = 2048
P = 128                    # partitions
M = BC // P                # rows per partition = 16
NCHUNK = 2                 # pipeline chunks
CHM = M // NCHUNK          # rows per partition per chunk = 8
F32 = mybir.dt.float32
F32R = mybir.dt.float32r
BF16 = mybir.dt.bfloat16
N_WARM = 8                 # junk matmuls to warm the PE clock gate


def _build_nc(neg3invt):
    nc = bacc.Bacc("TRN2", target_bir_lowering=False, debug=False,
                   num_devices=N_CORES)
    xd = nc.dram_tensor("x", [P, M * 6], F32, kind="ExternalInput")
    cstd = nc.dram_tensor("cst", [P, 2 * 24], F32, kind="ExternalInput")
    s2d = nc.dram_tensor("s2", [256, 160], BF16, kind="ExternalInput")
    idd = nc.dram_tensor("ident", [P, P], F32R, kind="ExternalInput")
    od = nc.dram_tensor("o", [P, M * 10], F32, kind="ExternalOutput")

    with tile.TileContext(nc) as tc:
        with tc.tile_pool(name="consts", bufs=1) as consts, \
             tc.tile_pool(name="work", bufs=2) as work, \
             tc.tile_pool(name="atp", bufs=8) as atp, \
             tc.tile_pool(name="ps_t", bufs=4, space="PSUM") as ps_t, \
             tc.tile_pool(name="ps_c", bufs=4, space="PSUM") as ps_c:
            # Junk matmuls on a memset tile keep the PE busy from the very
            # start of the kernel (before inputs land) so the HAM clock gate
            # is fully open when the real matmuls arrive.  fp32 (double-pass)
            # on purpose: more PE-busy cycles per instruction.  They rotate
            # through the multi-buffer transpose PSUM pool so they run dense
            # back-to-back (a single-buffer WAR chain leaves ~300ns gaps,
            # which keeps the clock gate closed AND delays the real work).
            junk = consts.tile([P, P], F32)
            nc.gpsimd.memset(junk[:], 1.0)
            for _ in range(N_WARM):
                wps = ps_t.tile([P, 2, P], F32R, tag="tp")
                nc.tensor.matmul(wps[:, 0, :].bitcast(F32), lhsT=junk[:],
                                 rhs=junk[:], start=True, stop=True)
            x_st = consts.tile([P, M, 6], F32)
            nc.sync.dma_start(out=x_st[:], in_=xd[:].rearrange("p (i f) -> p i f", i=M))
            cst_st = consts.tile([P, 2, 6, 4], F32)
            nc.scalar.dma_start(out=cst_st[:].rearrange("p k f j -> p (k f j)"),
                                in_=cstd[:])
            s2_sb = consts.tile([P, 2, 160], BF16)
            nc.scalar.dma_start(out=s2_sb[:], in_=s2d[:].rearrange("(k p) n -> p k n", p=P))
            ident = consts.tile([P, P], F32R)
            nc.scalar.dma_start(out=ident[:], in_=idd[:])

            # Phase 1 (both chunks): H build, exp, krons, normalizer, A.
            # Emitting ALL of it before any pair loop puts chunk 1's A early
            # in the DVE stream, so the PE runs both chunks' transpose/matmul
            # streams back-to-back and every cpp tile is ready before the
            # DVE's v-fold pass needs it (the PE has slack even at half
            # clock, so the HAM gate no longer matters).
            p45s, As = [], []
            for c in range(NCHUNK):
                xv = x_st[:, c * CHM:(c + 1) * CHM, :]
                # m = 3/t * relu(x) on ACT (Relu with positive scale commutes)
                m3 = work.tile([P, CHM, 6], F32, tag="m3")
                nc.scalar.activation(m3[:].rearrange("p i f -> p (i f)"),
                                     xv.rearrange("p i f -> p (i f)"),
                                     mybir.ActivationFunctionType.Relu,
                                     scale=-neg3invt)
                H = work.tile([P, CHM, 6, 4], F32, tag="H")
                nc.vector.tensor_mul(H[:], xv[:, :, :, None].broadcast_to((P, CHM, 6, 4)),
                                     cst_st[:, 0:1, :, :].broadcast_to((P, CHM, 6, 4)))
                nc.vector.tensor_add(H[:], H[:],
                                     cst_st[:, 1:2, :, :].broadcast_to((P, CHM, 6, 4)))
                nc.vector.tensor_sub(H[:], H[:],
                                     m3[:, :, :, None].broadcast_to((P, CHM, 6, 4)))
                E = work.tile([P, CHM, 6, 4], F32, tag="E")
                nc.scalar.activation(E[:].rearrange("p i f j -> p (i f j)"),
                                     H[:].rearrange("p i f j -> p (i f j)"),
                                     mybir.ActivationFunctionType.Exp)

                # p01/p23 krons (TensorTensor APs are limited to 3 free dims,
                # so the two feature-pairs can't fuse into one instruction)
                pairs = work.tile([P, CHM, 2, 16], F32, tag="pairs")
                for g, (fa, fb) in enumerate(((0, 1), (2, 3))):
                    nc.vector.tensor_mul(
                        pairs[:, :, g, :].rearrange("p i (a b) -> p i a b", a=4),
                        E[:, :, fa, :, None].broadcast_to((P, CHM, 4, 4)),
                        E[:, :, fb, None, :].broadcast_to((P, CHM, 4, 4)))
                p01 = pairs[:, :, 0, :]
                p23 = pairs[:, :, 1, :]
                # softmax normalizer: zr = 1/prod_f(sum_j E), folded into
                # bins4 before the p45 kron
                Z = work.tile([P, CHM, 6], F32, tag="Z")
                nc.vector.tensor_reduce(Z[:], E[:], axis=mybir.AxisListType.X,
                                        op=mybir.AluOpType.add)
                zp = work.tile([P, CHM], F32, tag="zp")
                nc.vector.tensor_reduce(zp[:], Z[:], axis=mybir.AxisListType.X,
                                        op=mybir.AluOpType.mult)
                zr = work.tile([P, CHM, 1], F32, tag="zr")
                nc.vector.reciprocal(zr[:, :, 0], zp[:])
                E4z = work.tile([P, CHM, 4], F32, tag="E4z")
                nc.vector.tensor_mul(E4z[:], E[:, :, 4, :],
                                     zr[:].broadcast_to((P, CHM, 4)))
                p45 = work.tile([P, CHM, 1, 16], F32, tag="p45")
                nc.vector.tensor_mul(
                    p45[:, :, 0, :].rearrange("p i (a b) -> p i a b", a=4),
                    E4z[:, :, :, None].broadcast_to((P, CHM, 4, 4)),
                    E[:, :, 5, None, :].broadcast_to((P, CHM, 4, 4)))
                A = work.tile([P, CHM, 256], F32R, tag="A")
                nc.vector.tensor_mul(
                    A[:].rearrange("p i (a b) -> p i a b", a=16),
                    p01[:, :, :, None].broadcast_to((P, CHM, 16, 16)),
                    p23[:, :, None, :].broadcast_to((P, CHM, 16, 16)))
                p45s.append(p45)
                As.append(A)

            # Phase 2 (both chunks): transpose / matmul / v-fold / store.
            for c in range(NCHUNK):
                A, p45 = As[c], p45s[c]
                D = work.tile([P, CHM, 10, 16], F32, tag="D")
                O = work.tile([P, CHM, 10], F32, tag="O")
                for pair in range(CHM // 2):
                    sl = slice(pair * 2, pair * 2 + 2)
                    cpp = ps_c.tile([P, 2, 160], F32, tag="cp")
                    for hhalf in range(2):
                        i = pair * 2 + hhalf
                        tp = ps_t.tile([P, 2, P], F32R, tag="tp")
                        for k in range(2):
                            nc.tensor.transpose(tp[:, k, :],
                                                A[:, i, k * P:(k + 1) * P],
                                                ident[:])
                        at2 = atp.tile([P, 2, P], BF16, tag="at")
                        nc.scalar.copy(out=at2[:], in_=tp[:].bitcast(F32))
                        nc.tensor.matmul(cpp[:, hhalf, :], lhsT=at2[:, 0, :],
                                         rhs=s2_sb[:, 0, :], start=True, stop=False)
                        nc.tensor.matmul(cpp[:, hhalf, :], lhsT=at2[:, 1, :],
                                         rhs=s2_sb[:, 1, :], start=False, stop=True)
                    nc.vector.tensor_mul(
                        D[:, sl],
                        cpp[:].rearrange("p i (c v) -> p i c v", c=10),
                        p45[:, sl].broadcast_to((P, 2, 10, 16)))
                    nc.vector.tensor_reduce(O[:, sl], D[:, sl],
                                            axis=mybir.AxisListType.X,
                                            op=mybir.AluOpType.add)
                hc = CHM // 2
                for hx in range(2):
                    nc.sync.dma_start(
                        out=od[:].rearrange("p (i c) -> p i c", i=M)
                        [:, c * CHM + hx * hc:c * CHM + (hx + 1) * hc, :],
                        in_=O[:, hx * hc:(hx + 1) * hc, :])
    nc.compile()
    return nc


_CACHE = {}


def _prep_inputs(x, cuts, leaf_score, temperature):
    import ml_dtypes
    x = np.ascontiguousarray(np.asarray(x, dtype=np.float32))
    cuts = np.asarray(cuts, dtype=np.float32)
    leaf_score = np.asarray(leaf_score, dtype=np.float32)
    invt = 1.0 / float(np.asarray(temperature).reshape(-1)[0])

    # host-side parameter prep (tiny)
    sc = np.sort(cuts, axis=1)
    bias = np.cumsum(np.concatenate([np.zeros((6, 1), np.float32), -sc], axis=1,
                                    dtype=np.float32), axis=1)          # [6,4]
    W = np.arange(1.0, 5.0, dtype=np.float32)
    w2 = np.tile(((W - 1.0) * invt)[None, :], (6, 1))                    # [6,4]
    bt = bias * invt                                                     # [6,4]
    cst = np.ascontiguousarray(np.broadcast_to(
        np.stack([w2, bt]).reshape(1, 48), (P, 48)).astype(np.float32))
    s2 = np.ascontiguousarray(
        leaf_score.reshape(256, 16, 10).transpose(0, 2, 1).reshape(256, 160)
    ).astype(ml_dtypes.bfloat16)
    ident = np.eye(P, dtype=np.float32)

    xs = x.reshape(N_CORES, P, M * 6)
    return invt, [{"x": xs[i], "cst": cst, "s2": s2, "ident": ident}
                  for i in range(N_CORES)]


def kernel(x, cuts, leaf_score, temperature):
    invt, in_maps = _prep_inputs(x, cuts, leaf_score, temperature)
    key = ("nc", float(invt))
    if key not in _CACHE:
        _CACHE[key] = _build_nc(-3.0 * invt)
        _CACHE["nc"] = _CACHE[key]
    nc = _CACHE[key]
    res = run_bass_kernel_spmd(nc, in_maps, list(range(N_CORES))).results
    out = np.concatenate([r["o"].reshape(BC, 10) for r in res], axis=0)
    return out.astype(np.float32)
